# revision 39
# baseline (speedup 1.0000x reference)
"""Trainium2 Bass kernel: masked causal attention (sparse_attention).

Reference computation (B=4, Tq=Tv=4096, D=64, fp32):
    scores = einsum("bqd,bkd->bqk", Q, K) * scale
    mask   = v_mask[:,None,:] & tril            (causal & key mask)
    scores = scores - 1e9 * ~mask
    out    = softmax(scores, -1) @ V
    out   *= q_mask[...,None]

Sharding: 2 cores per batch; the pair splits keys by alternating 128-wide
k-blocks (even/odd interleave).  Every core runs an identical program over
8 query superblocks of 512; causal pruning keeps only k-blocks at-or-below
the diagonal, and the even/odd interleave gives both parities the same
tile counts, so one SPMD program serves all 8 cores (only data differs).

Math notes:
 - softmax max-subtraction is skipped: |scores| <~ 50 so exp() stays well
   inside fp32 range; result is mathematically identical.
 - v_mask is folded into V' = [V*m | m] on the host: the PV matmul then
   yields both the masked numerator and the masked denominator (row 64).
 - causal masking inside the two diagonal-boundary tiles per superblock is
   a 0/1 multiply with per-core constant masks (data, not program).
 - rows whose causal window is fully v-masked reproduce the reference's
   fp32 "scores - 1e9" bucketing; fixed up on the host (only the first few
   rows of a batch can be affected).
 - matmuls run in float32r (PE full-rate fp32 mode, reduced mantissa).

Device per-core layout (S^T tiles [k=128 part, q=512 free]):
  QK:  S^T = kt_blk[64,128].T @ qt[64,512]      (PE)
  exp: P^T = exp(S^T)                           (ACT, PSUM->SBUF, 2-bank
       groups to amortize the per-instruction overhead)
  PV:  oT[65,512] += vp_blk[128,65].T @ P^T     (PE, PSUM accumulate)
Host: sum parity partials, out = (oT[:64]/oT[64]).T, q_mask, edge fixup.
"""

import sys

import numpy as np

try:  # concourse ships on the container's site path; fall back to the repo
    import concourse  # noqa: F401
except ImportError:  # pragma: no cover
    sys.path.insert(0, "/opt/trn_rl_repo")

B, T, D = 4, 4096, 64
NCORES = 8
QS_N = 8          # query superblocks per batch
QSB = 512         # superblock width
KB = 128          # k-block width
GRP = 2           # k-blocks per PSUM group (2 banks)
NEG_BIG = 1e9

_compiled = None


def _build_nc():
    import concourse.bass as bass
    import concourse.mybir as mybir
    import concourse.tile as tile
    from concourse import bacc

    f32 = mybir.dt.float32
    f32r = mybir.dt.float32r
    bf16 = mybir.dt.bfloat16
    nc = bacc.Bacc(None, target_bir_lowering=False, debug=False)

    qt_d = nc.declare_dram_parameter("qt", [D, T], f32r, isOutput=False)
    kt_d = nc.declare_dram_parameter("kt", [D, T // 2], f32r, isOutput=False)
    vp_d = nc.declare_dram_parameter("vp", [KB, 16 * 65], f32r, isOutput=False)
    mk_d = nc.declare_dram_parameter("mk", [KB, QSB + 256], bf16, isOutput=False)
    o_d = nc.declare_dram_parameter("o", [65, T], f32, isOutput=True)

    with tile.TileContext(nc) as tc:
        with (
            tc.tile_pool(name="const", bufs=1) as cpool,
            tc.tile_pool(name="pt", bufs=6) as ppool,
            tc.tile_pool(name="ps", bufs=3, space=bass.MemorySpace.PSUM) as spool,
            tc.tile_pool(name="po", bufs=2, space=bass.MemorySpace.PSUM) as opool,
        ):
            qt = cpool.tile([D, T], f32r)
            kt = cpool.tile([D, T // 2], f32r)
            vp = cpool.tile([KB, 16 * 65], f32r)
            # mk[k, q'] = (q' >= 256 + par*128 + k); its two overlapping
            # 512-wide windows are the boundary masks for kb = nkb-2 / nkb-1
            mk = cpool.tile([KB, QSB + 256], bf16)
            # loads ordered for the (0, 1, 7, 6, ...) superblock schedule:
            # the qs=0/1 prefix first, then everything qs=7 needs, the rest
            nc.sync.dma_start(qt[:, 0:2 * QSB], qt_d[:, 0:2 * QSB])
            nc.sync.dma_start(kt[:, 0:4 * KB], kt_d[:, 0:4 * KB])
            nc.sync.dma_start(qt[:, 7 * QSB:T], qt_d[:, 7 * QSB:T])
            nc.sync.dma_start(kt[:, 4 * KB:T // 2], kt_d[:, 4 * KB:T // 2])
            nc.sync.dma_start(mk[:], mk_d[:])
            nc.sync.dma_start(vp[:], vp_d[:])
            nc.sync.dma_start(qt[:, 2 * QSB:7 * QSB], qt_d[:, 2 * QSB:7 * QSB])

            # flat list of (qs, group) work items; PV of item i-1 is
            # emitted between QK(i) and exp(i) consumers so the PE always has
            # fresh QK output ready for ACT, and ACT output ready for PV.
            items = []
            for qs in (0, 1, 7, 6, 5, 4, 3, 2):
                nkb = 2 * qs + 2
                ngrp = (nkb + GRP - 1) // GRP
                for g in range(ngrp):
                    gkbs = list(range(g * GRP, min((g + 1) * GRP, nkb)))
                    if qs > 0 and gkbs[-1] == nkb - 1:
                        gkbs = gkbs[::-1]
                    items.append((qs, g, nkb, gkbs))

            o_of = {}
            pending = None  # (qs, gkbs, nkb, p_tile)

            def emit_pv(qs, gkbs, nkb, p):
                last_grp = nkb - 1 in gkbs
                for j, kb in enumerate(gkbs):
                    c0 = 256 if kb == nkb - 1 and kb != 0 else 0
                    nc.tensor.matmul(
                        o_of[qs][:, c0:QSB],
                        vp[:, kb * 65:(kb + 1) * 65],
                        p[:, j * QSB + c0:(j + 1) * QSB],
                        start=(kb == 0),
                        stop=(last_grp and j == len(gkbs) - 1),
                    )
                if last_grp:  # superblock finished
                    q0 = qs * QSB
                    ob = obpool.tile([65, QSB], f32, name=f"ob{qs}",
                                     tag="ob")
                    nc.vector.tensor_copy(ob[:], o_of[qs][:])
                    nc.sync.dma_start(o_d[:, q0:q0 + QSB], ob[:])
                    del o_of[qs]

            for qs, g, nkb, gkbs in items:
                if g == 0:
                    o_of[qs] = opool.tile([65, QSB], f32, name=f"oacc{qs}", tag="o_acc")
                q0 = qs * QSB
                w = len(gkbs) * QSB
                rev = gkbs[0] > gkbs[-1]  # reversed boundary group
                s = spool.tile([KB, GRP * QSB], f32)
                for j, kb in enumerate(gkbs):
                    c0 = 256 if rev and j == 0 else 0
                    nc.tensor.matmul(
                        s[:, j * QSB + c0:(j + 1) * QSB],
                        kt[:, kb * KB:(kb + 1) * KB],
                        qt[:, q0 + c0:q0 + QSB],
                        start=True, stop=True,
                    )
                if pending is not None:
                    emit_pv(*pending)
                e0 = 256 if rev else 0
                p = ppool.tile([KB, GRP * QSB], f32r)
                nc.scalar.activation(
                    p[:, e0:w], s[:, e0:w],
                    mybir.ActivationFunctionType.Exp,
                )
                # causal masks on the diagonal-boundary k-blocks.
                # mk here is the fused layout F = [Bwin(256) | Awin(512)]:
                # for a reversed group both boundary blocks are adjacent in
                # p ([256:1024]) and F matches it exactly -> one DVE op.
                if rev and nkb - 2 in gkbs:
                    nc.vector.tensor_tensor(
                        p[:, 256:2 * QSB],
                        p[:, 256:2 * QSB],
                        mk[:, 0:QSB + 256],
                        op=mybir.AluOpType.mult,
                    )
                else:
                    for kb, f0, fw, c0 in ((nkb - 2, 256, QSB, 0),
                                           (nkb - 1, 0, 256, 256)):
                        if kb in gkbs:
                            j = gkbs.index(kb)
                            nc.vector.tensor_tensor(
                                p[:, j * QSB + c0:(j + 1) * QSB],
                                p[:, j * QSB + c0:(j + 1) * QSB],
                                mk[:, f0:f0 + fw],
                                op=mybir.AluOpType.mult,
                            )
                pending = (qs, gkbs, nkb, p)
            emit_pv(*pending)

    nc.compile()
    return nc


def _get_nc():
    global _compiled
    if _compiled is None:
        _compiled = _build_nc()
    return _compiled


def _host_inputs(query, value, keys, q_mask, v_mask, scale):
    """Build the 8 per-core input maps."""
    scale = np.float32(scale)
    q = np.asarray(query, np.float32)
    v = np.asarray(value, np.float32)
    k = np.asarray(keys, np.float32)
    vm = np.asarray(v_mask).astype(np.float32)

    kk = np.arange(KB)[:, None]
    qq = np.arange(QSB)[None, :]
    in_maps = []
    for c in range(NCORES):
        b, par = c // 2, c % 2
        qt = np.ascontiguousarray(q[b].T * scale)            # [64, 4096]
        # even/odd k-block interleave -> 16 local blocks of 128
        kblk = k[b].reshape(32, KB, D)[par::2]               # [16,128,64]
        kt = np.ascontiguousarray(
            kblk.reshape(16 * KB, D).T)                      # [64, 2048]
        vprime = np.concatenate(
            [v[b] * vm[b][:, None], vm[b][:, None]], axis=1)  # [4096, 65]
        vblk = vprime.reshape(32, KB, 65)[par::2]            # [16,128,65]
        vp = np.ascontiguousarray(
            vblk.transpose(1, 0, 2).reshape(KB, 16 * 65))    # [128, 1040]
        # causal masks for the two boundary tiles: keep q_f >= j*128 + k_p
        import ml_dtypes
        qq7 = np.arange(QSB + 256)[None, :]
        mk_plain = (qq7 >= 256 + par * KB + kk)
        # fused layout: [B window (old mk[:,256:512]) | A window (old
        # mk[:,256:768])]
        mk = np.concatenate(
            [mk_plain[:, 256:512], mk_plain[:, 256:768]], axis=1
        ).astype(ml_dtypes.bfloat16)
        in_maps.append({"qt": qt, "kt": kt, "vp": vp,
                        "mk": np.ascontiguousarray(mk)})
    return in_maps


def _host_gather(results, query, value, keys, q_mask, v_mask, scale):
    q = np.asarray(query, np.float32)
    v = np.asarray(value, np.float32)
    k = np.asarray(keys, np.float32)
    qm = np.asarray(q_mask).astype(bool)
    vm = np.asarray(v_mask).astype(bool)
    scale = np.float32(scale)

    out = np.empty((B, T, D), np.float32)
    for b in range(B):
        oT = results[2 * b]["o"] + results[2 * b + 1]["o"]   # [65, 4096]
        l = oT[64]
        out[b] = (oT[:64] / np.where(l > 0, l, 1.0)).T
        # rows whose whole causal window is v-masked: reference degenerates
        # to softmax over ALL raw scores (every entry got the same -1e9).
        nz = np.flatnonzero(vm[b])
        first = nz[0] if nz.size else T
        if first > 0:
            rows = np.arange(first)
            # every score of these rows is masked, so the reference's fp32
            # `s - 1e9` collapses scores into 64-wide buckets (fp32 ulp at
            # 1e9); softmax is over those bucketed values.  Replicate in
            # fp32 exactly, then stable softmax in fp64.
            s = ((q[b, rows] @ k[b].T) * scale).astype(np.float32)
            s = s - np.float32(NEG_BIG)
            s = s.astype(np.float64)
            s -= s.max(axis=1, keepdims=True)
            p = np.exp(s)
            p /= p.sum(axis=1, keepdims=True)
            out[b, rows] = p @ v[b].astype(np.float64)
    out = np.where(qm[..., None], out, np.float32(0.0))
    return out


def kernel(**inputs):
    from concourse.bass_utils import run_bass_kernel_spmd

    nc = _get_nc()
    in_maps = _host_inputs(**inputs)
    res = run_bass_kernel_spmd(nc, in_maps, list(range(NCORES))).results
    return _host_gather(res, **inputs)


if __name__ == "__main__":
    rng = np.random.default_rng(0)
    inputs = {
        "query": rng.standard_normal((B, T, D)).astype(np.float32),
        "value": rng.standard_normal((B, T, D)).astype(np.float32),
        "keys": rng.standard_normal((B, T, D)).astype(np.float32),
        "q_mask": rng.integers(0, 2, (B, T)) > 0,
        "v_mask": rng.integers(0, 2, (B, T)) > 0,
        "scale": np.float32(1.0),
    }
    out = kernel(**inputs)
    print(out.shape, out.dtype)
